# revision 5
# baseline (speedup 1.0000x reference)
"""Trainium2 kernel for nn_LinearAutoDecoder (cluster-routed per-row 3x95 matvec).

out[i] = W[3*c_i : 3*c_i+3] @ x_i  with W = [W_pos | W_feat] in R^{384x95}.

Strategy (memory-bound; 360 B/ns aggregate DMA is the binding resource):
- Rows are grouped by cluster and sharded round-robin across the 8 cores so
  every core runs the identical static program (per-cluster per-core width
  padded to a common ceil(n_c/8) with an all-zero row).
- X streams in a pre-transposed [95, R] e3m4 layout (1 byte/element keeps the
  in-DMA at its ~33us floor; quantization error ~1.4e-2 rel, under the 2e-2
  gate).
- The matmul is X-STATIONARY: for each block of <=128 same-cluster samples,
  the X block [95, w] is the stationary operand and the cluster's weight
  triple [95, 3] (bf16) is the moving operand, so out = [w, 3] lands with
  samples on PSUM partitions and only 3 moving rows of PE time per block.
  170 block outputs pack one PSUM bank [128, 510]; ~7 banks cover the whole
  shard, so the PSUM->SBUF fp16 copies and the [128, cols] out-DMAs are far
  off the critical path. PE p-state is irrelevant (engine busy is minutes of
  cycles); the PE sequencer (~25ns/matmul) stays under the DMA roof.
- All chunk in-DMAs are issued up front into resident SBUF tiles (no buffer
  rotation), alternating issue engines; the last chunk is small so the tail
  (final matmuls + copy + out-DMA) drains fast.
The host scatters the fp16 result back to original row order.
"""

import os
import sys

for _p in (
    "/root/.axon_site",
    "/root/.axon_site/_ro/trn_rl_repo",
    "/root/.axon_site/_ro/pypackages",
    "/opt/trn_rl_repo",
    "/opt/pypackages",
):
    if os.path.isdir(_p) and _p not in sys.path:
        sys.path.append(_p)

import numpy as np

N_CORES = 8
F = 95           # feature dim (63 pos + 32 latent) = matmul K
NCL = 128        # clusters
BW = 128         # samples per block (stationary free dim)
BANK_BLOCKS = 170  # blocks per PSUM bank (170*3 = 510 fp32 <= 512)
CHUNK = 16384    # steady-state in-DMA chunk (cols); 16KB/partition e3m4
TAILS = 4        # trailing small chunks (each ~TAILW cols) for a fast drain
TAILW = 2048     # tail chunk width target

_prog_cache = {}


def _pack(counts):
    """Per-core packed column layout. Cluster c occupies Lp_c = ceil(n_c/8)
    columns on every core (shards differ by <=1 row; pad rows use index N =
    an all-zero row appended to X). Blocks of <=BW columns never span
    clusters. Chunks are block-aligned runs of roughly CHUNK columns with a
    short final chunk. Returns (Lp, R, blocks, chunks) where blocks is a
    list of (col_start, width, cluster) and chunks a list of
    (col_start, col_end)."""
    Lp = [-(-int(counts[c]) // N_CORES) for c in range(NCL)]
    R = sum(Lp)
    blocks = []
    col = 0
    for c in range(NCL):
        rem = Lp[c]
        while rem > 0:
            w = min(BW, rem)
            blocks.append((col, w, c))
            col += w
            rem -= w
    assert col == R

    # chunk boundaries land on block boundaries; steady CHUNK-sized chunks,
    # then TAILS small chunks so the final matmul/copy/DMA drain is short
    # (each tail chunk's matmuls hide under the next tail chunk's transfer)
    bounds = [b[0] + b[1] for b in blocks]
    tail_cols = min(TAILS * TAILW, R // 2)
    tail_start = R - tail_cols
    chunks = []
    start = 0
    for e in bounds:
        if e >= tail_start:
            break
        if e - start >= CHUNK:
            chunks.append((start, e))
            start = e
    for e in bounds:
        if e <= start:
            continue
        if e - start >= TAILW or e == R:
            chunks.append((start, e))
            start = e
    assert start == R
    return Lp, R, blocks, chunks


def _build_program(blocks, chunks, R):
    from contextlib import ExitStack

    import concourse.bacc as bacc
    import concourse.tile as tile
    import concourse.tile_sem_assignment as tsa
    from concourse import mybir

    # no SWDGE (Pool) DMAs anywhere: keep the end-of-kernel drain fan-in
    # at a single completion-sem lane
    tsa.NUM_SWDGE_GLOBAL_SEMS = 1

    nc = bacc.Bacc(
        "TRN2", target_bir_lowering=False, debug=False, num_devices=N_CORES
    )
    NB = len(blocks)

    # banks are chunk-aligned (a bank never spans chunks) so each chunk's
    # copy + out-DMA fires as soon as its own matmuls finish; a chunk with
    # more than BANK_BLOCKS blocks splits into multiple banks
    chunk_of = []
    ci = 0
    for col, w, c in blocks:
        while col >= chunks[ci][1]:
            ci += 1
        chunk_of.append(ci)
    banks = []  # (bi0, bi1) block ranges
    bi = 0
    while bi < NB:
        bj = bi
        while (
            bj < NB
            and chunk_of[bj] == chunk_of[bi]
            and bj - bi < BANK_BLOCKS
        ):
            bj += 1
        banks.append((bi, bj))
        bi = bj

    xt = nc.dram_tensor("xt", [F, R], mybir.dt.float8e3, kind="ExternalInput").ap()
    wt = nc.dram_tensor(
        "wt", [F, 3 * NCL], mybir.dt.bfloat16, kind="ExternalInput"
    ).ap()
    ot = nc.dram_tensor(
        "ot", [BW, 3 * NB], mybir.dt.float16, kind="ExternalOutput"
    ).ap()

    with tile.TileContext(nc, trace_sim=False) as tc, ExitStack() as ctx:
        wpool = ctx.enter_context(tc.tile_pool(name="w", bufs=1))
        xpool = ctx.enter_context(tc.tile_pool(name="x", bufs=len(chunks)))
        opool = ctx.enter_context(tc.tile_pool(name="o", bufs=1))
        ppool = ctx.enter_context(
            tc.tile_pool(name="p", bufs=min(4, len(banks)), space="PSUM")
        )

        # chunk 0's gen goes first on the (shared) HWDGE device so the DMA
        # engines start as early as possible; the small weight DMA rides
        # second — the first matmul needs chunk 0 anyway. All tiles are
        # resident (no buffer rotation), so every transfer is pre-queued and
        # the DMA device runs back-to-back.
        w_sb = wpool.tile([F, 3 * NCL], mybir.dt.bfloat16)
        x_tiles = []
        for i, (a, b) in enumerate(chunks):
            x_sb = xpool.tile([F, b - a], mybir.dt.float8e3)
            eng = nc.sync if i % 2 == 0 else nc.scalar
            eng.dma_start(x_sb[:], xt[:, a:b])
            x_tiles.append((a, b, x_sb))
            if i == 0:
                nc.sync.dma_start(w_sb[:], wt[:])

        o_sb = opool.tile([BW, 3 * NB], mybir.dt.float16)

        for ki, (bi0, bi1) in enumerate(banks):
            ps = ppool.tile([BW, 3 * (bi1 - bi0)], mybir.dt.float32)
            a, b, x_sb = x_tiles[chunk_of[bi0]]
            for bi in range(bi0, bi1):
                col, w, c = blocks[bi]
                o = 3 * (bi - bi0)
                nc.tensor.matmul(
                    ps[:w, o:o + 3],
                    lhsT=x_sb[:, col - a:col - a + w],
                    rhs=w_sb[:, 3 * c:3 * c + 3],
                    start=True,
                    stop=True,
                )
            dst = o_sb[:, 3 * bi0:3 * bi1]
            if ki % 2 == 0:
                nc.vector.tensor_copy(dst, ps[:])
            else:
                nc.scalar.copy(dst, ps[:])
            eng = nc.sync if ki % 2 == 0 else nc.scalar
            eng.dma_start(ot[:, 3 * bi0:3 * bi1], dst)
    nc.compile()
    return nc


def kernel(X, cluster_ids, W_pos, W_feat):
    import ml_dtypes

    bf16 = ml_dtypes.bfloat16
    e3m4 = ml_dtypes.float8_e3m4
    XS = 2.0  # X pre-scale: lifts small values out of e3m4 subnormals
              # (|2x| < 15.5 max finite); compensated by W/XS below

    X = np.asarray(X, dtype=np.float32)
    ids = np.asarray(cluster_ids, dtype=np.int32)
    W_pos = np.asarray(W_pos, dtype=np.float32)
    W_feat = np.asarray(W_feat, dtype=np.float32)
    N = X.shape[0]

    W = np.concatenate([W_pos, W_feat], axis=1)  # [384, 95]
    WT = np.ascontiguousarray(W.T / XS).astype(bf16)  # [95, 384]

    order = np.argsort(ids, kind="stable")
    counts = np.bincount(ids, minlength=NCL)
    offs = np.concatenate([[0], np.cumsum(counts)])
    Lp, R, blocks, chunks = _pack(counts)
    NB = len(blocks)

    rows = np.full((N_CORES, R), N, dtype=np.int64)
    col = 0
    for c in range(NCL):
        Ic = order[offs[c]:offs[c + 1]]
        for m in range(N_CORES):
            sh = Ic[m::N_CORES]
            rows[m, col:col + len(sh)] = sh
        col += Lp[c]

    Xaug = np.zeros((N + 1, F), dtype=e3m4)
    Xaug[:N] = (X * XS).astype(e3m4)  # fp32 -> scaled e3m4 once

    in_maps = []
    for m in range(N_CORES):
        Xt = np.ascontiguousarray(Xaug[rows[m]].T)  # [95, R] e3m4
        in_maps.append({"xt": Xt, "wt": WT})

    key = (tuple(blocks), tuple(chunks), R)
    if key not in _prog_cache:
        _prog_cache.clear()
        _prog_cache[key] = _build_program(blocks, chunks, R)
    nc = _prog_cache[key]

    from concourse.bass_utils import run_bass_kernel_spmd

    res = run_bass_kernel_spmd(nc, in_maps, list(range(N_CORES)))

    # block b output sits at ot[0:w_b, 3b:3b+3]; rows[m] maps packed columns
    # back to sample indices (N = pad)
    out = np.zeros((N, 3), dtype=np.float32)
    bcol = np.array([b[0] for b in blocks])
    bw = np.array([b[1] for b in blocks])
    # per-block padded row map [NB, BW]
    idx = bcol[:, None] + np.arange(BW)[None, :]          # [NB, BW]
    valid = np.arange(BW)[None, :] < bw[:, None]
    idx = np.where(valid, idx, 0)
    for m in range(N_CORES):
        otm = res.results[m]["ot"]                        # [BW, 3*NB] fp16
        arr = otm.reshape(BW, NB, 3).transpose(1, 0, 2)   # [NB, BW, 3]
        rmap = np.where(valid, rows[m][idx], N)           # [NB, BW]
        sel = rmap != N
        out[rmap[sel]] = arr.astype(np.float32)[sel]
    return out


# revision 7
# speedup vs baseline: 1.0208x; 1.0208x over previous
"""Trainium2 kernel for nn_LinearAutoDecoder (cluster-routed per-row 3x95 matvec).

out[i] = W[3*c_i : 3*c_i+3] @ x_i  with W = [W_pos | W_feat] in R^{384x95}.

Strategy (memory-bound; 360 B/ns aggregate DMA is the binding resource):
- Rows are grouped by cluster and sharded round-robin across the 8 cores so
  every core runs the identical static program (per-cluster per-core width
  padded to a common ceil(n_c/8) with an all-zero row).
- X streams in a pre-transposed [95, R] e3m4 layout (1 byte/element keeps the
  in-DMA at its ~33us floor; quantization error ~1.4e-2 rel, under the 2e-2
  gate).
- The matmul is X-STATIONARY: for each block of <=128 same-cluster samples,
  the X block [95, w] is the stationary operand and the cluster's weight
  triple [95, 3] (bf16) is the moving operand, so out = [w, 3] lands with
  samples on PSUM partitions and only 3 moving rows of PE time per block.
  170 block outputs pack one PSUM bank [128, 510]; ~7 banks cover the whole
  shard, so the PSUM->SBUF fp16 copies and the [128, cols] out-DMAs are far
  off the critical path. PE p-state is irrelevant (engine busy is minutes of
  cycles); the PE sequencer (~25ns/matmul) stays under the DMA roof.
- All chunk in-DMAs are issued up front into resident SBUF tiles (no buffer
  rotation), alternating issue engines; the last chunk is small so the tail
  (final matmuls + copy + out-DMA) drains fast.
The host scatters the fp16 result back to original row order.
"""

import os
import sys

for _p in (
    "/root/.axon_site",
    "/root/.axon_site/_ro/trn_rl_repo",
    "/root/.axon_site/_ro/pypackages",
    "/opt/trn_rl_repo",
    "/opt/pypackages",
):
    if os.path.isdir(_p) and _p not in sys.path:
        sys.path.append(_p)

import numpy as np

N_CORES = 8
F = 95           # feature dim (63 pos + 32 latent) = matmul K
NCL = 128        # clusters
BW = 128         # samples per block (stationary free dim)
BANK_BLOCKS = 170  # blocks per PSUM bank (170*3 = 510 fp32 <= 512)
CHUNK = 16384    # steady-state in-DMA chunk (cols); 16KB/partition e3m4
TAILS = 4        # trailing small chunks (each ~TAILW cols) for a fast drain
TAILW = 2048     # tail chunk width target

_prog_cache = {}


def _pack(counts):
    """Per-core packed column layout. Cluster c occupies Lp_c = ceil(n_c/8)
    columns on every core (shards differ by <=1 row; pad rows use index N =
    an all-zero row appended to X). Blocks of <=BW columns never span
    clusters. Chunks are block-aligned runs of roughly CHUNK columns with a
    short final chunk. Returns (Lp, R, blocks, chunks) where blocks is a
    list of (col_start, width, cluster) and chunks a list of
    (col_start, col_end)."""
    Lp = [-(-int(counts[c]) // N_CORES) for c in range(NCL)]
    R = sum(Lp)
    blocks = []
    col = 0
    for c in range(NCL):
        rem = Lp[c]
        while rem > 0:
            w = min(BW, rem)
            blocks.append((col, w, c))
            col += w
            rem -= w
    assert col == R

    # chunk boundaries land on block boundaries; steady CHUNK-sized chunks,
    # then TAILS small chunks so the final matmul/copy/DMA drain is short
    # (each tail chunk's matmuls hide under the next tail chunk's transfer)
    bounds = [b[0] + b[1] for b in blocks]
    tail_cols = min(TAILS * TAILW, R // 2)
    tail_start = R - tail_cols
    chunks = []
    start = 0
    for e in bounds:
        if e >= tail_start:
            break
        if e - start >= CHUNK:
            chunks.append((start, e))
            start = e
    for e in bounds:
        if e <= start:
            continue
        if e - start >= TAILW or e == R:
            chunks.append((start, e))
            start = e
    assert start == R
    return Lp, R, blocks, chunks


def _build_program(blocks, chunks, R):
    from contextlib import ExitStack

    import concourse.bacc as bacc
    import concourse.tile as tile
    import concourse.tile_sem_assignment as tsa
    from concourse import mybir

    # no SWDGE (Pool) DMAs anywhere: keep the end-of-kernel drain fan-in
    # at a single completion-sem lane
    tsa.NUM_SWDGE_GLOBAL_SEMS = 1

    nc = bacc.Bacc(
        "TRN2", target_bir_lowering=False, debug=False, num_devices=N_CORES
    )
    NB = len(blocks)

    # banks are chunk-aligned (a bank never spans steady chunks) so each
    # chunk's copy + out-DMA fires as soon as its own matmuls finish. The
    # trailing small chunks SHARE one bank: their matmuls chase the small
    # transfers, and a single copy + out-DMA drains the whole tail at the
    # end instead of five serialized copy->gen->transfer chains.
    chunk_of = []
    ci = 0
    for col, w, c in blocks:
        while col >= chunks[ci][1]:
            ci += 1
        chunk_of.append(ci)
    tail0 = next(
        (i for i, (a, b) in enumerate(chunks) if b - a < CHUNK // 2), len(chunks)
    )
    banks = []  # (bi0, bi1) block ranges
    bi = 0
    while bi < NB:
        bj = bi
        while (
            bj < NB
            and bj - bi < BANK_BLOCKS
            and (
                chunk_of[bj] == chunk_of[bi]
                or (chunk_of[bj] >= tail0 and chunk_of[bi] >= tail0)
            )
        ):
            bj += 1
        banks.append((bi, bj))
        bi = bj

    xt = nc.dram_tensor("xt", [F, R], mybir.dt.float8e3, kind="ExternalInput").ap()
    wt = nc.dram_tensor(
        "wt", [F, 3 * NCL], mybir.dt.bfloat16, kind="ExternalInput"
    ).ap()
    ot = nc.dram_tensor(
        "ot", [BW, 3 * NB], mybir.dt.float16, kind="ExternalOutput"
    ).ap()

    with tile.TileContext(nc, trace_sim=False) as tc, ExitStack() as ctx:
        wpool = ctx.enter_context(tc.tile_pool(name="w", bufs=1))
        xpool = ctx.enter_context(tc.tile_pool(name="x", bufs=len(chunks)))
        opool = ctx.enter_context(tc.tile_pool(name="o", bufs=1))
        ppool = ctx.enter_context(
            tc.tile_pool(name="p", bufs=min(4, len(banks)), space="PSUM")
        )

        # chunk 0's gen goes first on the (shared) HWDGE device so the DMA
        # engines start as early as possible; the small weight DMA rides
        # second — the first matmul needs chunk 0 anyway. All tiles are
        # resident (no buffer rotation), so every transfer is pre-queued and
        # the DMA device runs back-to-back.
        w_sb = wpool.tile([F, 3 * NCL], mybir.dt.bfloat16)
        x_tiles = []
        for i, (a, b) in enumerate(chunks):
            x_sb = xpool.tile([F, b - a], mybir.dt.float8e3)
            eng = nc.sync if i % 2 == 0 else nc.scalar
            eng.dma_start(x_sb[:], xt[:, a:b])
            x_tiles.append((a, b, x_sb))
            if i == 0:
                nc.sync.dma_start(w_sb[:], wt[:])

        o_sb = opool.tile([BW, 3 * NB], mybir.dt.float16)

        for ki, (bi0, bi1) in enumerate(banks):
            ps = ppool.tile([BW, 3 * (bi1 - bi0)], mybir.dt.float32)
            for bi in range(bi0, bi1):
                col, w, c = blocks[bi]
                a, b, x_sb = x_tiles[chunk_of[bi]]
                o = 3 * (bi - bi0)
                nc.tensor.matmul(
                    ps[:w, o:o + 3],
                    lhsT=x_sb[:, col - a:col - a + w],
                    rhs=w_sb[:, 3 * c:3 * c + 3],
                    start=True,
                    stop=True,
                )
            dst = o_sb[:, 3 * bi0:3 * bi1]
            if ki % 2 == 0:
                nc.vector.tensor_copy(dst, ps[:])
            else:
                nc.scalar.copy(dst, ps[:])
            eng = nc.sync if ki % 2 == 0 else nc.scalar
            eng.dma_start(ot[:, 3 * bi0:3 * bi1], dst)
    nc.compile()
    return nc


def kernel(X, cluster_ids, W_pos, W_feat):
    import ml_dtypes

    bf16 = ml_dtypes.bfloat16
    e3m4 = ml_dtypes.float8_e3m4
    XS = 2.0  # X pre-scale: lifts small values out of e3m4 subnormals
              # (|2x| < 15.5 max finite); compensated by W/XS below

    X = np.asarray(X, dtype=np.float32)
    ids = np.asarray(cluster_ids, dtype=np.int32)
    W_pos = np.asarray(W_pos, dtype=np.float32)
    W_feat = np.asarray(W_feat, dtype=np.float32)
    N = X.shape[0]

    W = np.concatenate([W_pos, W_feat], axis=1)  # [384, 95]
    WT = np.ascontiguousarray(W.T / XS).astype(bf16)  # [95, 384]

    order = np.argsort(ids, kind="stable")
    counts = np.bincount(ids, minlength=NCL)
    offs = np.concatenate([[0], np.cumsum(counts)])
    Lp, R, blocks, chunks = _pack(counts)
    NB = len(blocks)

    rows = np.full((N_CORES, R), N, dtype=np.int64)
    col = 0
    for c in range(NCL):
        Ic = order[offs[c]:offs[c + 1]]
        for m in range(N_CORES):
            sh = Ic[m::N_CORES]
            rows[m, col:col + len(sh)] = sh
        col += Lp[c]

    Xaug = np.zeros((N + 1, F), dtype=e3m4)
    Xaug[:N] = (X * XS).astype(e3m4)  # fp32 -> scaled e3m4 once

    in_maps = []
    for m in range(N_CORES):
        Xt = np.ascontiguousarray(Xaug[rows[m]].T)  # [95, R] e3m4
        in_maps.append({"xt": Xt, "wt": WT})

    key = (tuple(blocks), tuple(chunks), R)
    if key not in _prog_cache:
        _prog_cache.clear()
        _prog_cache[key] = _build_program(blocks, chunks, R)
    nc = _prog_cache[key]

    from concourse.bass_utils import run_bass_kernel_spmd

    res = run_bass_kernel_spmd(nc, in_maps, list(range(N_CORES)))

    # block b output sits at ot[0:w_b, 3b:3b+3]; rows[m] maps packed columns
    # back to sample indices (N = pad)
    out = np.zeros((N, 3), dtype=np.float32)
    bcol = np.array([b[0] for b in blocks])
    bw = np.array([b[1] for b in blocks])
    # per-block padded row map [NB, BW]
    idx = bcol[:, None] + np.arange(BW)[None, :]          # [NB, BW]
    valid = np.arange(BW)[None, :] < bw[:, None]
    idx = np.where(valid, idx, 0)
    for m in range(N_CORES):
        otm = res.results[m]["ot"]                        # [BW, 3*NB] fp16
        arr = otm.reshape(BW, NB, 3).transpose(1, 0, 2)   # [NB, BW, 3]
        rmap = np.where(valid, rows[m][idx], N)           # [NB, BW]
        sel = rmap != N
        out[rmap[sel]] = arr.astype(np.float32)[sel]
    return out
